# revision 43
# baseline (speedup 1.0000x reference)
"""MinGRU forward on 8 TRN2 NeuronCores.

Math (linear-space reformulation of the reference's log-space Heinsen scan):
    hg = x @ W_hg.T                       # [B,S,2D]
    hidden, gate = split(hg)
    z = sigmoid(gate)
    c = sigmoid(-gate)                    # = 1 - z = exp(-softplus(gate))
    g = max(hidden + 0.5, sigmoid(hidden))  # == where(h>=0, h+0.5, sigmoid(h)) exactly
    u = z * g
    h[t] = c[t] * h[t-1] + u[t]           # convex combination -> bounded, stable
    out = h

The recurrence maps onto the DVE `tensor_tensor_scan` (fp32 internal state).

Sharding: 8 cores = 4 batches x 2 feature-halves (512 features each).
No cross-core communication: the scan is per-feature independent.

Performance structure (measured on HW, ~134us vs 150.7us fp32r baseline):
- fp16 inputs (10-bit mantissa ~ fp32r's 11) stream at the PE's full rate
  (216ns per 512-col matmul) with half the HBM bytes of fp32r.
- Host pre-packs x/W/out into chunk-blocked layouts so every DMA row is a
  2-8KB contiguous run: DGE queues crawl on sub-2KB rows but hit ~350GB/s
  when backlogged with large-row descriptors.
- Chunks 0-1 are fc-STAGED (fc0/fc1 over both chunks, then fc2/fc3) so the
  DMA ramp only needs half of W early; pieces land in PE consumption order
  across the three HWDGE queues. Outputs alternate between the GpSimd and
  SP queues so they never block x streaming and drain in parallel at the
  end. h/g matmuls interleave per k to halve the boundary arrival rate.
- Elementwise runs in fp16: sigmoid(h)/h+0.5/z/c on the scalar engine
  straight from PSUM (psum recycles without waiting on the DVE), g/u on
  the DVE in its 2x 16-bit mode, scan in fp16 with fp32 internal state.
  Output fp16, upcast on host (tolerance 2e-2; measured error 2.8e-3).
- The tail after the last matmul is fixed-cost bound (~0.3-0.5us per op),
  so the final 128-token chunk groups fc0/fc1 into shared psum with wide
  elementwise ops, leaving only fc3's narrow chain at the very end.
"""

import numpy as np

B, S, D = 4, 4096, 1024
DH = D // 2          # features per core
N_CORES = 8
KC = 128             # contraction chunk (partition dim)
NKC = D // KC        # 8 k chunks
FC = 128             # feature chunk (psum partitions)
NFC = DH // FC       # 4 feature chunks

_CACHE = {}

# Chunk widths: 512 (one PSUM bank) in steady state, narrow at the end to
# shorten the serial post-matmul tail (act -> scan -> out-DMA).
WIDTHS = [512, 512, 512, 512, 512, 512, 512, 384, 128]
assert sum(WIDTHS) == S
W2 = 2 * FC          # weight columns per fc (hidden 128 | gate 128)


def _build():
    import concourse.bacc as bacc
    import concourse.tile as tile
    import concourse.mybir as mybir

    f32 = mybir.dt.float32
    f16 = mybir.dt.float16
    AF = mybir.ActivationFunctionType
    OP = mybir.AluOpType

    nc = bacc.Bacc("TRN2")
    # const AP for the activation bias (+0.5); only 0.0/1.0 are pre-registered.
    # The memset runs in the preamble, ~7us before the first activation that
    # reads it; gpsimd program order also puts it before any gpsimd DMA.
    const_half = nc.alloc_sbuf_tensor("const-float32-0.5", [128, 1], f32)
    nc.vector.memset(const_half.ap(), 0.5)
    nc.const_aps.aps[(f32, 0.5)] = const_half.ap()
    # Chunk-blocked SBUF images (host does the shuffles):
    #   xQ[p, NKC*off(sc) + k*w + j] = x[b][off+j, k*128+p]
    #   wQ[fc, p, k*256 + e]         = W row (c*DH + fc*128 + e%128), col k*128+p
    #   outQ[DH*off(sc) + (fc*128+p)*w + j] = h[off+j, fc*128+p]
    xQ = nc.dram_tensor("xQ", [KC, NKC * S], f16, kind="ExternalInput")
    wQ = nc.dram_tensor("wQ", [NFC, KC, NKC * W2], f16, kind="ExternalInput")
    outQ = nc.dram_tensor("outQ", [DH * S], f16, kind="ExternalOutput")

    with tile.TileContext(nc) as tc:
        with (
            tc.tile_pool(name="w", bufs=1) as wpool,
            tc.tile_pool(name="x", bufs=3) as xpool,
            tc.tile_pool(name="ew", bufs=3) as epool,
            tc.tile_pool(name="h", bufs=2) as hpool,
            tc.tile_pool(name="ps", bufs=4, space="PSUM") as pspool,
        ):
            wts = []
            for fc in range(NFC):
                wtf = wpool.tile([KC, NKC * W2], f16, tag=f"w{fc}")
                wts.append(wtf)

            # Ramp notes (measured): DMA pieces with <2KB rows crawl; each
            # queue pays ~2us startup + ~0.6us issue per DMA; scalar's queue
            # is separate from sync's and starts early; gpsimd's starts
            # ~1.7us later; a backlogged queue moves ~350GB/s.
            w0 = WIDTHS[0]
            xt0 = xpool.tile([KC, NKC * w0], f16, tag="xt")

            def wpiece(eng, fc, k0, k1):  # k-range of W fc (quads: 2KB rows)
                eng.dma_start(wts[fc][:, k0 * W2:k1 * W2],
                              wQ[fc, :, k0 * W2:k1 * W2])

            def xpair(eng, k):
                eng.dma_start(xt0[:, k * w0:(k + 2) * w0],
                              xQ[:, k * w0:(k + 2) * w0])

            # Chunks 0-1 run fc-STAGED (fc0/fc1 over both chunks, then
            # fc2/fc3), so the early window only needs half of W: the ramp
            # is never W-supply-bound. Early needs: x0 + Wfc0 now, Wfc1 at
            # +3.5us, xt1 at +7us; everything else has >10us of slack.
            wpiece(nc.sync, 0, 0, 4)   # first W quad on the earliest queue
            for k in range(0, NKC, 2):
                xpair(nc.sync, k)
            wpiece(nc.scalar, 0, 4, 8)
            wpiece(nc.scalar, 1, 0, 8)
            xt1 = xpool.tile([KC, NKC * WIDTHS[1]], f16, tag="xt")
            nc.sync.dma_start(xt1[:], xQ[:, NKC * w0:NKC * (w0 + WIDTHS[1])])
            wpiece(nc.sync, 2, 0, 8)
            wpiece(nc.gpsimd, 3, 0, 8)

            hprev = [None] * NFC
            offs = []
            o = 0
            for w in WIDTHS:
                offs.append(o)
                o += w
            xts = {0: xt0, 1: xt1}

            def sched():
                yield from ((0, 0), (0, 1), (1, 0), (1, 1),
                            (0, 2), (0, 3), (1, 2), (1, 3))
                for sc_ in range(2, len(WIDTHS) - 1):
                    for fc_ in range(NFC):
                        yield (sc_, fc_)

            for sc, fc in sched():
                width, off = WIDTHS[sc], offs[sc]
                if sc not in xts:
                    xt = xpool.tile([KC, NKC * width], f16, tag="xt")
                    base = NKC * off
                    nc.sync.dma_start(xt[:], xQ[:, base:base + NKC * width])
                    xts[sc] = xt
                xt = xts[sc]
                ph = pspool.tile([FC, width], f32, tag="ph")
                pg = pspool.tile([FC, width], f32, tag="pg")
                # h/g interleaved per k: same steady cadence as split loops,
                # but halves the W/x arrival-rate requirement at boundaries
                for k in range(NKC):
                    nc.tensor.matmul(
                        ph[:], wts[fc][:, k * W2:k * W2 + FC],
                        xt[:, k * width:(k + 1) * width],
                        start=(k == 0), stop=(k == NKC - 1),
                    )
                    nc.tensor.matmul(
                        pg[:], wts[fc][:, k * W2 + FC:(k + 1) * W2],
                        xt[:, k * width:(k + 1) * width],
                        start=(k == 0), stop=(k == NKC - 1),
                    )
                st = epool.tile([FC, width], f16, tag="s")
                hp = epool.tile([FC, width], f16, tag="hp")
                zt = epool.tile([FC, width], f16, tag="z")
                ct = epool.tile([FC, width], f16, tag="c")
                gt = epool.tile([FC, width], f16, tag="g")
                ut = epool.tile([FC, width], f16, tag="u")
                # scalar engine drains psum: s/hp from ph, z/c from pg
                nc.scalar.activation(st[:], ph[:], AF.Sigmoid)
                nc.scalar.activation(hp[:], ph[:], AF.Identity, bias=0.5)
                nc.scalar.activation(zt[:], pg[:], AF.Sigmoid)
                nc.scalar.activation(ct[:], pg[:], AF.Sigmoid, scale=-1.0)
                # DVE in 2x fp16 mode: g = max(h+0.5, sigmoid(h)); u = z*g
                nc.vector.tensor_tensor(gt[:], hp[:], st[:], op=OP.max)
                nc.vector.tensor_mul(ut[:], zt[:], gt[:])
                ht = hpool.tile([FC, width], f16, tag=f"h{fc}")
                init = 0.0 if sc == 0 else hprev[fc][:, WIDTHS[sc - 1] - 1:WIDTHS[sc - 1]]
                nc.vector.tensor_tensor_scan(
                    ht[:], ct[:], ut[:], init, op0=OP.mult, op1=OP.add
                )
                hprev[fc] = ht
                dst = outQ[
                    DH * off + fc * FC * width:
                    DH * off + (fc + 1) * FC * width
                ].rearrange("(p w) -> p w", p=FC)
                # alternate queues so the final chunks' outputs drain in
                # parallel at the end instead of serializing on one ring
                oeng = nc.gpsimd if sc % 2 == 0 else nc.sync
                oeng.dma_start(dst, ht[:])

            # ---- last chunk ----
            for sc in (len(WIDTHS) - 1,):
                width, off = WIDTHS[sc], offs[sc]
                xt = xpool.tile([KC, NKC * width], f16, tag="xt")
                base = NKC * off
                nc.sync.dma_start(xt[:], xQ[:, base:base + NKC * width])
                if True:
                    # Last chunk (width 128): the tail after the final matmul
                    # is dominated by per-op fixed costs (~0.3-0.5us each).
                    # Group fc0-2 into one shared psum pair with triple-width
                    # elementwise ops (overlapping fc3's matmuls), leaving
                    # only fc3's minimal narrow chain after the last matmul.
                    pw = WIDTHS[sc - 1]
                    for gfcs in ((0, 1), (2,), (3,)):
                        gw = len(gfcs) * width
                        phm = pspool.tile([FC, gw], f32, tag="ph")
                        pgm = pspool.tile([FC, gw], f32, tag="pg")
                        for gi, fc in enumerate(gfcs):
                            sl = slice(gi * width, (gi + 1) * width)
                            for k in range(NKC):
                                nc.tensor.matmul(
                                    phm[:, sl], wts[fc][:, k * W2:k * W2 + FC],
                                    xt[:, k * width:(k + 1) * width],
                                    start=(k == 0), stop=(k == NKC - 1),
                                )
                                nc.tensor.matmul(
                                    pgm[:, sl], wts[fc][:, k * W2 + FC:(k + 1) * W2],
                                    xt[:, k * width:(k + 1) * width],
                                    start=(k == 0), stop=(k == NKC - 1),
                                )
                        st = epool.tile([FC, gw], f16, tag="s")
                        hp = epool.tile([FC, gw], f16, tag="hp")
                        zt = epool.tile([FC, gw], f16, tag="z")
                        ct = epool.tile([FC, gw], f16, tag="c")
                        gt = epool.tile([FC, gw], f16, tag="g")
                        ut = epool.tile([FC, gw], f16, tag="u")
                        nc.scalar.activation(st[:], phm[:], AF.Sigmoid)
                        nc.scalar.activation(hp[:], phm[:], AF.Identity, bias=0.5)
                        nc.scalar.activation(zt[:], pgm[:], AF.Sigmoid)
                        nc.scalar.activation(ct[:], pgm[:], AF.Sigmoid, scale=-1.0)
                        nc.vector.tensor_tensor(gt[:], hp[:], st[:], op=OP.max)
                        nc.vector.tensor_mul(ut[:], zt[:], gt[:])
                        for gi, fc in enumerate(gfcs):
                            sl = slice(gi * width, (gi + 1) * width)
                            ht = hpool.tile([FC, width], f16, tag=f"h{fc}")
                            nc.vector.tensor_tensor_scan(
                                ht[:], ct[:, sl], ut[:, sl],
                                hprev[fc][:, pw - 1:pw],
                                op0=OP.mult, op1=OP.add,
                            )
                            dst = outQ[
                                DH * off + fc * FC * width:
                                DH * off + (fc + 1) * FC * width
                            ].rearrange("(p w) -> p w", p=FC)
                            oeng = nc.gpsimd if fc % 2 == 0 else nc.sync
                            oeng.dma_start(dst, ht[:])

    nc.compile()
    return nc


def _prep_in_maps(x: np.ndarray, W_hg: np.ndarray):
    x = np.asarray(x, dtype=np.float32)
    W_hg = np.asarray(W_hg, dtype=np.float32)
    xQs = []
    for b in range(B):
        xb = x[b].astype(np.float16)                        # [S, D]
        xq = np.empty((KC, NKC * S), dtype=np.float16)
        o = 0
        for w in WIDTHS:
            blk = xb[o:o + w].T.reshape(NKC, KC, w).transpose(1, 0, 2)
            xq[:, NKC * o:NKC * (o + w)] = blk.reshape(KC, NKC * w)
            o += w
        xQs.append(xq)
    wQs = []
    for c in range(2):
        wq = np.empty((NFC, KC, NKC * W2), dtype=np.float16)
        for fc in range(NFC):
            rows_h = W_hg[c * DH + fc * FC:c * DH + (fc + 1) * FC]      # [FC, D]
            rows_g = W_hg[D + c * DH + fc * FC:D + c * DH + (fc + 1) * FC]
            wfc = np.empty((D, W2), dtype=np.float16)
            wfc[:, 0:FC] = rows_h.T
            wfc[:, FC:W2] = rows_g.T
            wq[fc] = wfc.reshape(NKC, KC, W2).transpose(1, 0, 2).reshape(KC, NKC * W2)
        wQs.append(wq)
    return [{"xQ": xQs[core // 2], "wQ": wQs[core % 2]} for core in range(N_CORES)]


def _get_runner():
    """Build the Bass module once and cache a compiled jax callable for it."""
    if "runner" in _CACHE:
        return _CACHE["runner"]

    import jax
    from jax.experimental.shard_map import shard_map
    from jax.sharding import Mesh, PartitionSpec
    from concourse import bass2jax

    if "nc" not in _CACHE:
        _CACHE["nc"] = _build()
    nc = _CACHE["nc"]
    bass2jax.install_neuronx_cc_hook()

    in_names = ["xQ", "wQ"]
    out_name = "outQ"
    out_shape, out_dtype = (DH * S,), np.float16
    partition_name = nc.partition_id_tensor.name if nc.partition_id_tensor else None

    def _body(xQ, wQ, zout):
        operands = [xQ, wQ, zout]
        if partition_name is not None:
            operands.append(bass2jax.partition_id_tensor())
        outs = bass2jax._bass_exec_p.bind(
            *operands,
            out_avals=(jax.core.ShapedArray(out_shape, out_dtype),),
            in_names=tuple(in_names + [out_name] + ([partition_name] if partition_name else [])),
            out_names=(out_name,),
            lowering_input_output_aliases=(),
            sim_require_finite=True,
            sim_require_nnan=True,
            nc=nc,
        )
        return tuple(outs)

    devices = jax.devices()[:N_CORES]
    mesh = Mesh(np.asarray(devices), ("core",))
    sharded = jax.jit(
        shard_map(
            _body, mesh=mesh,
            in_specs=(PartitionSpec("core"),) * 3,
            out_specs=(PartitionSpec("core"),),
            check_rep=False,
        ),
        donate_argnums=(2,),
        keep_unused=True,
    )

    def run(in_maps):
        concat_x = np.concatenate([m["xQ"] for m in in_maps], axis=0)
        concat_w = np.concatenate([m["wQ"] for m in in_maps], axis=0)
        zeros = np.zeros((N_CORES * DH * S,), np.float16)
        (out_arr,) = sharded(concat_x, concat_w, zeros)
        return np.asarray(out_arr).reshape(N_CORES, DH * S)

    _CACHE["runner"] = run
    return run


def kernel(x: np.ndarray, W_hg: np.ndarray) -> np.ndarray:
    run = _get_runner()
    in_maps = _prep_in_maps(x, W_hg)
    outs = run(in_maps)

    out = np.empty((B, S, D), dtype=np.float32)
    for core in range(N_CORES):
        b, c = core // 2, core % 2
        flat = outs[core]
        o = 0
        for w in WIDTHS:
            blk = flat[DH * o:DH * (o + w)].reshape(DH, w)
            out[b, o:o + w, c * DH:(c + 1) * DH] = blk.T.astype(np.float32)
            o += w
    return out


# revision 44
# speedup vs baseline: 1.0071x; 1.0071x over previous
"""MinGRU forward on 8 TRN2 NeuronCores.

Math (linear-space reformulation of the reference's log-space Heinsen scan):
    hg = x @ W_hg.T                       # [B,S,2D]
    hidden, gate = split(hg)
    z = sigmoid(gate)
    c = sigmoid(-gate)                    # = 1 - z = exp(-softplus(gate))
    g = max(hidden + 0.5, sigmoid(hidden))  # == where(h>=0, h+0.5, sigmoid(h)) exactly
    u = z * g
    h[t] = c[t] * h[t-1] + u[t]           # convex combination -> bounded, stable
    out = h

The recurrence maps onto the DVE `tensor_tensor_scan` (fp32 internal state).

Sharding: 8 cores = 4 batches x 2 feature-halves (512 features each).
No cross-core communication: the scan is per-feature independent.

Performance structure (measured on HW, ~134us vs 150.7us fp32r baseline):
- fp16 inputs (10-bit mantissa ~ fp32r's 11) stream at the PE's full rate
  (216ns per 512-col matmul) with half the HBM bytes of fp32r.
- Host pre-packs x/W/out into chunk-blocked layouts so every DMA row is a
  2-8KB contiguous run: DGE queues crawl on sub-2KB rows but hit ~350GB/s
  when backlogged with large-row descriptors.
- Chunks 0-1 are fc-STAGED (fc0/fc1 over both chunks, then fc2/fc3) so the
  DMA ramp only needs half of W early; pieces land in PE consumption order
  across the three HWDGE queues. Outputs alternate between the GpSimd and
  SP queues so they never block x streaming and drain in parallel at the
  end. h/g matmuls interleave per k to halve the boundary arrival rate.
- Elementwise runs in fp16: sigmoid(h)/h+0.5/z/c on the scalar engine
  straight from PSUM (psum recycles without waiting on the DVE), g/u on
  the DVE in its 2x 16-bit mode, scan in fp16 with fp32 internal state.
  Output fp16, upcast on host (tolerance 2e-2; measured error 2.8e-3).
- The tail after the last matmul is fixed-cost bound (~0.3-0.5us per op),
  so the final 128-token chunk groups fc0/fc1 into shared psum with wide
  elementwise ops, leaving only fc3's narrow chain at the very end.
"""

import numpy as np

B, S, D = 4, 4096, 1024
DH = D // 2          # features per core
N_CORES = 8
KC = 128             # contraction chunk (partition dim)
NKC = D // KC        # 8 k chunks
FC = 128             # feature chunk (psum partitions)
NFC = DH // FC       # 4 feature chunks

_CACHE = {}

# Chunk widths: 512 (one PSUM bank) in steady state, narrow at the end to
# shorten the serial post-matmul tail (act -> scan -> out-DMA).
WIDTHS = [512, 512, 512, 512, 512, 512, 512, 384, 128]
assert sum(WIDTHS) == S
W2 = 2 * FC          # weight columns per fc (hidden 128 | gate 128)


def _build():
    import concourse.bacc as bacc
    import concourse.tile as tile
    import concourse.mybir as mybir

    f32 = mybir.dt.float32
    f16 = mybir.dt.float16
    AF = mybir.ActivationFunctionType
    OP = mybir.AluOpType

    nc = bacc.Bacc("TRN2")
    # const AP for the activation bias (+0.5); only 0.0/1.0 are pre-registered.
    # The memset runs in the preamble, ~7us before the first activation that
    # reads it; gpsimd program order also puts it before any gpsimd DMA.
    const_half = nc.alloc_sbuf_tensor("const-float32-0.5", [128, 1], f32)
    nc.vector.memset(const_half.ap(), 0.5)
    nc.const_aps.aps[(f32, 0.5)] = const_half.ap()
    # Chunk-blocked SBUF images (host does the shuffles):
    #   xQ[p, NKC*off(sc) + k*w + j] = x[b][off+j, k*128+p]
    #   wQ[fc, p, k*256 + e]         = W row (c*DH + fc*128 + e%128), col k*128+p
    #   outQ[DH*off(sc) + (fc*128+p)*w + j] = h[off+j, fc*128+p]
    xQ = nc.dram_tensor("xQ", [KC, NKC * S], f16, kind="ExternalInput")
    wQ = nc.dram_tensor("wQ", [NFC, KC, NKC * W2], f16, kind="ExternalInput")
    outQ = nc.dram_tensor("outQ", [DH * S], f16, kind="ExternalOutput")

    with tile.TileContext(nc) as tc:
        with (
            tc.tile_pool(name="w", bufs=1) as wpool,
            tc.tile_pool(name="x", bufs=3) as xpool,
            tc.tile_pool(name="ew", bufs=3) as epool,
            tc.tile_pool(name="h", bufs=2) as hpool,
            tc.tile_pool(name="ps", bufs=4, space="PSUM") as pspool,
        ):
            wts = []
            for fc in range(NFC):
                wtf = wpool.tile([KC, NKC * W2], f16, tag=f"w{fc}")
                wts.append(wtf)

            # Ramp notes (measured): DMA pieces with <2KB rows crawl; each
            # queue pays ~2us startup + ~0.6us issue per DMA; scalar's queue
            # is separate from sync's and starts early; gpsimd's starts
            # ~1.7us later; a backlogged queue moves ~350GB/s.
            w0 = WIDTHS[0]
            xt0 = xpool.tile([KC, NKC * w0], f16, tag="xt")

            def wpiece(eng, fc, k0, k1):  # k-range of W fc (quads: 2KB rows)
                eng.dma_start(wts[fc][:, k0 * W2:k1 * W2],
                              wQ[fc, :, k0 * W2:k1 * W2])

            def xpair(eng, k):
                eng.dma_start(xt0[:, k * w0:(k + 2) * w0],
                              xQ[:, k * w0:(k + 2) * w0])

            # Chunks 0-1 run fc-STAGED (fc0/fc1 over both chunks, then
            # fc2/fc3), so the early window only needs half of W: the ramp
            # is never W-supply-bound. Early needs: x0 + Wfc0 now, Wfc1 at
            # +3.5us, xt1 at +7us; everything else has >10us of slack.
            for k in range(0, NKC, 2):
                xpair(nc.sync, k)
            wpiece(nc.scalar, 0, 0, 4)
            wpiece(nc.scalar, 0, 4, 8)
            wpiece(nc.scalar, 1, 0, 8)
            xt1 = xpool.tile([KC, NKC * WIDTHS[1]], f16, tag="xt")
            nc.sync.dma_start(xt1[:], xQ[:, NKC * w0:NKC * (w0 + WIDTHS[1])])
            wpiece(nc.sync, 2, 0, 8)
            wpiece(nc.gpsimd, 3, 0, 8)

            hprev = [None] * NFC
            offs = []
            o = 0
            for w in WIDTHS:
                offs.append(o)
                o += w
            xts = {0: xt0, 1: xt1}

            def sched():
                yield from ((0, 0), (0, 1), (1, 0), (1, 1),
                            (0, 2), (0, 3), (1, 2), (1, 3))
                for sc_ in range(2, len(WIDTHS) - 1):
                    for fc_ in range(NFC):
                        yield (sc_, fc_)

            for sc, fc in sched():
                width, off = WIDTHS[sc], offs[sc]
                if sc not in xts:
                    xt = xpool.tile([KC, NKC * width], f16, tag="xt")
                    base = NKC * off
                    nc.sync.dma_start(xt[:], xQ[:, base:base + NKC * width])
                    xts[sc] = xt
                xt = xts[sc]
                ph = pspool.tile([FC, width], f32, tag="ph")
                pg = pspool.tile([FC, width], f32, tag="pg")
                # h/g interleaved per k: same steady cadence as split loops,
                # but halves the W/x arrival-rate requirement at boundaries
                for k in range(NKC):
                    nc.tensor.matmul(
                        ph[:], wts[fc][:, k * W2:k * W2 + FC],
                        xt[:, k * width:(k + 1) * width],
                        start=(k == 0), stop=(k == NKC - 1),
                    )
                    nc.tensor.matmul(
                        pg[:], wts[fc][:, k * W2 + FC:(k + 1) * W2],
                        xt[:, k * width:(k + 1) * width],
                        start=(k == 0), stop=(k == NKC - 1),
                    )
                st = epool.tile([FC, width], f16, tag="s")
                hp = epool.tile([FC, width], f16, tag="hp")
                zt = epool.tile([FC, width], f16, tag="z")
                ct = epool.tile([FC, width], f16, tag="c")
                gt = epool.tile([FC, width], f16, tag="g")
                ut = epool.tile([FC, width], f16, tag="u")
                # scalar engine drains psum: s/hp from ph, z/c from pg
                nc.scalar.activation(st[:], ph[:], AF.Sigmoid)
                nc.scalar.activation(hp[:], ph[:], AF.Identity, bias=0.5)
                nc.scalar.activation(zt[:], pg[:], AF.Sigmoid)
                # c = 1 - z on the DVE (2x fp16): frees pg after one read and
                # drops the 4th scalar activation from every tail chain
                nc.vector.tensor_scalar(ct[:], zt[:], -1.0, 1.0,
                                        op0=OP.mult, op1=OP.add)
                # DVE in 2x fp16 mode: g = max(h+0.5, sigmoid(h)); u = z*g
                nc.vector.tensor_tensor(gt[:], hp[:], st[:], op=OP.max)
                nc.vector.tensor_mul(ut[:], zt[:], gt[:])
                ht = hpool.tile([FC, width], f16, tag=f"h{fc}")
                init = 0.0 if sc == 0 else hprev[fc][:, WIDTHS[sc - 1] - 1:WIDTHS[sc - 1]]
                nc.vector.tensor_tensor_scan(
                    ht[:], ct[:], ut[:], init, op0=OP.mult, op1=OP.add
                )
                hprev[fc] = ht
                dst = outQ[
                    DH * off + fc * FC * width:
                    DH * off + (fc + 1) * FC * width
                ].rearrange("(p w) -> p w", p=FC)
                # alternate queues so the final chunks' outputs drain in
                # parallel at the end instead of serializing on one ring
                oeng = nc.gpsimd if sc % 2 == 0 else nc.sync
                oeng.dma_start(dst, ht[:])

            # ---- last chunk ----
            for sc in (len(WIDTHS) - 1,):
                width, off = WIDTHS[sc], offs[sc]
                xt = xpool.tile([KC, NKC * width], f16, tag="xt")
                base = NKC * off
                nc.sync.dma_start(xt[:], xQ[:, base:base + NKC * width])
                if True:
                    # Last chunk (width 128): the tail after the final matmul
                    # is dominated by per-op fixed costs (~0.3-0.5us each).
                    # Group fc0-2 into one shared psum pair with triple-width
                    # elementwise ops (overlapping fc3's matmuls), leaving
                    # only fc3's minimal narrow chain after the last matmul.
                    pw = WIDTHS[sc - 1]
                    for gfcs in ((0, 1), (2,), (3,)):
                        gw = len(gfcs) * width
                        phm = pspool.tile([FC, gw], f32, tag="ph")
                        pgm = pspool.tile([FC, gw], f32, tag="pg")
                        for gi, fc in enumerate(gfcs):
                            sl = slice(gi * width, (gi + 1) * width)
                            for k in range(NKC):
                                nc.tensor.matmul(
                                    phm[:, sl], wts[fc][:, k * W2:k * W2 + FC],
                                    xt[:, k * width:(k + 1) * width],
                                    start=(k == 0), stop=(k == NKC - 1),
                                )
                                nc.tensor.matmul(
                                    pgm[:, sl], wts[fc][:, k * W2 + FC:(k + 1) * W2],
                                    xt[:, k * width:(k + 1) * width],
                                    start=(k == 0), stop=(k == NKC - 1),
                                )
                        st = epool.tile([FC, gw], f16, tag="s")
                        hp = epool.tile([FC, gw], f16, tag="hp")
                        zt = epool.tile([FC, gw], f16, tag="z")
                        ct = epool.tile([FC, gw], f16, tag="c")
                        gt = epool.tile([FC, gw], f16, tag="g")
                        ut = epool.tile([FC, gw], f16, tag="u")
                        nc.scalar.activation(st[:], phm[:], AF.Sigmoid)
                        nc.scalar.activation(hp[:], phm[:], AF.Identity, bias=0.5)
                        nc.scalar.activation(zt[:], pgm[:], AF.Sigmoid)
                        nc.vector.tensor_scalar(ct[:], zt[:], -1.0, 1.0,
                                                op0=OP.mult, op1=OP.add)
                        nc.vector.tensor_tensor(gt[:], hp[:], st[:], op=OP.max)
                        nc.vector.tensor_mul(ut[:], zt[:], gt[:])
                        for gi, fc in enumerate(gfcs):
                            sl = slice(gi * width, (gi + 1) * width)
                            ht = hpool.tile([FC, width], f16, tag=f"h{fc}")
                            nc.vector.tensor_tensor_scan(
                                ht[:], ct[:, sl], ut[:, sl],
                                hprev[fc][:, pw - 1:pw],
                                op0=OP.mult, op1=OP.add,
                            )
                            dst = outQ[
                                DH * off + fc * FC * width:
                                DH * off + (fc + 1) * FC * width
                            ].rearrange("(p w) -> p w", p=FC)
                            oeng = nc.gpsimd if fc % 2 == 0 else nc.sync
                            oeng.dma_start(dst, ht[:])

    nc.compile()
    return nc


def _prep_in_maps(x: np.ndarray, W_hg: np.ndarray):
    x = np.asarray(x, dtype=np.float32)
    W_hg = np.asarray(W_hg, dtype=np.float32)
    xQs = []
    for b in range(B):
        xb = x[b].astype(np.float16)                        # [S, D]
        xq = np.empty((KC, NKC * S), dtype=np.float16)
        o = 0
        for w in WIDTHS:
            blk = xb[o:o + w].T.reshape(NKC, KC, w).transpose(1, 0, 2)
            xq[:, NKC * o:NKC * (o + w)] = blk.reshape(KC, NKC * w)
            o += w
        xQs.append(xq)
    wQs = []
    for c in range(2):
        wq = np.empty((NFC, KC, NKC * W2), dtype=np.float16)
        for fc in range(NFC):
            rows_h = W_hg[c * DH + fc * FC:c * DH + (fc + 1) * FC]      # [FC, D]
            rows_g = W_hg[D + c * DH + fc * FC:D + c * DH + (fc + 1) * FC]
            wfc = np.empty((D, W2), dtype=np.float16)
            wfc[:, 0:FC] = rows_h.T
            wfc[:, FC:W2] = rows_g.T
            wq[fc] = wfc.reshape(NKC, KC, W2).transpose(1, 0, 2).reshape(KC, NKC * W2)
        wQs.append(wq)
    return [{"xQ": xQs[core // 2], "wQ": wQs[core % 2]} for core in range(N_CORES)]


def _get_runner():
    """Build the Bass module once and cache a compiled jax callable for it."""
    if "runner" in _CACHE:
        return _CACHE["runner"]

    import jax
    from jax.experimental.shard_map import shard_map
    from jax.sharding import Mesh, PartitionSpec
    from concourse import bass2jax

    if "nc" not in _CACHE:
        _CACHE["nc"] = _build()
    nc = _CACHE["nc"]
    bass2jax.install_neuronx_cc_hook()

    in_names = ["xQ", "wQ"]
    out_name = "outQ"
    out_shape, out_dtype = (DH * S,), np.float16
    partition_name = nc.partition_id_tensor.name if nc.partition_id_tensor else None

    def _body(xQ, wQ, zout):
        operands = [xQ, wQ, zout]
        if partition_name is not None:
            operands.append(bass2jax.partition_id_tensor())
        outs = bass2jax._bass_exec_p.bind(
            *operands,
            out_avals=(jax.core.ShapedArray(out_shape, out_dtype),),
            in_names=tuple(in_names + [out_name] + ([partition_name] if partition_name else [])),
            out_names=(out_name,),
            lowering_input_output_aliases=(),
            sim_require_finite=True,
            sim_require_nnan=True,
            nc=nc,
        )
        return tuple(outs)

    devices = jax.devices()[:N_CORES]
    mesh = Mesh(np.asarray(devices), ("core",))
    sharded = jax.jit(
        shard_map(
            _body, mesh=mesh,
            in_specs=(PartitionSpec("core"),) * 3,
            out_specs=(PartitionSpec("core"),),
            check_rep=False,
        ),
        donate_argnums=(2,),
        keep_unused=True,
    )

    def run(in_maps):
        concat_x = np.concatenate([m["xQ"] for m in in_maps], axis=0)
        concat_w = np.concatenate([m["wQ"] for m in in_maps], axis=0)
        zeros = np.zeros((N_CORES * DH * S,), np.float16)
        (out_arr,) = sharded(concat_x, concat_w, zeros)
        return np.asarray(out_arr).reshape(N_CORES, DH * S)

    _CACHE["runner"] = run
    return run


def kernel(x: np.ndarray, W_hg: np.ndarray) -> np.ndarray:
    run = _get_runner()
    in_maps = _prep_in_maps(x, W_hg)
    outs = run(in_maps)

    out = np.empty((B, S, D), dtype=np.float32)
    for core in range(N_CORES):
        b, c = core // 2, core % 2
        flat = outs[core]
        o = 0
        for w in WIDTHS:
            blk = flat[DH * o:DH * (o + w)].reshape(DH, w)
            out[b, o:o + w, c * DH:(c + 1) * DH] = blk.T.astype(np.float32)
            o += w
    return out


# revision 45
# speedup vs baseline: 1.0140x; 1.0069x over previous
"""MinGRU forward on 8 TRN2 NeuronCores.

Math (linear-space reformulation of the reference's log-space Heinsen scan):
    hg = x @ W_hg.T                       # [B,S,2D]
    hidden, gate = split(hg)
    z = sigmoid(gate)
    c = sigmoid(-gate)                    # = 1 - z = exp(-softplus(gate))
    g = max(hidden + 0.5, sigmoid(hidden))  # == where(h>=0, h+0.5, sigmoid(h)) exactly
    u = z * g
    h[t] = c[t] * h[t-1] + u[t]           # convex combination -> bounded, stable
    out = h

The recurrence maps onto the DVE `tensor_tensor_scan` (fp32 internal state).

Sharding: 8 cores = 4 batches x 2 feature-halves (512 features each).
No cross-core communication: the scan is per-feature independent.

Performance structure (measured on HW, ~134us vs 150.7us fp32r baseline):
- fp16 inputs (10-bit mantissa ~ fp32r's 11) stream at the PE's full rate
  (216ns per 512-col matmul) with half the HBM bytes of fp32r.
- Host pre-packs x/W/out into chunk-blocked layouts so every DMA row is a
  2-8KB contiguous run: DGE queues crawl on sub-2KB rows but hit ~350GB/s
  when backlogged with large-row descriptors.
- Chunks 0-1 are fc-STAGED (fc0/fc1 over both chunks, then fc2/fc3) so the
  DMA ramp only needs half of W early; pieces land in PE consumption order
  across the three HWDGE queues. Outputs alternate between the GpSimd and
  SP queues so they never block x streaming and drain in parallel at the
  end. h/g matmuls interleave per k to halve the boundary arrival rate.
- Elementwise runs in fp16: sigmoid(h)/h+0.5/z/c on the scalar engine
  straight from PSUM (psum recycles without waiting on the DVE), g/u on
  the DVE in its 2x 16-bit mode, scan in fp16 with fp32 internal state.
  Output fp16, upcast on host (tolerance 2e-2; measured error 2.8e-3).
- The tail after the last matmul is fixed-cost bound (~0.3-0.5us per op),
  so the final 128-token chunk groups fc0/fc1 into shared psum with wide
  elementwise ops, leaving only fc3's narrow chain at the very end.
"""

import numpy as np

B, S, D = 4, 4096, 1024
DH = D // 2          # features per core
N_CORES = 8
KC = 128             # contraction chunk (partition dim)
NKC = D // KC        # 8 k chunks
FC = 128             # feature chunk (psum partitions)
NFC = DH // FC       # 4 feature chunks

_CACHE = {}

# Chunk widths: 512 (one PSUM bank) in steady state, narrow at the end to
# shorten the serial post-matmul tail (act -> scan -> out-DMA).
WIDTHS = [512, 512, 512, 512, 512, 512, 512, 384, 128]
assert sum(WIDTHS) == S
W2 = 2 * FC          # weight columns per fc (hidden 128 | gate 128)


def _build():
    import concourse.bacc as bacc
    import concourse.tile as tile
    import concourse.mybir as mybir

    f32 = mybir.dt.float32
    f16 = mybir.dt.float16
    AF = mybir.ActivationFunctionType
    OP = mybir.AluOpType

    nc = bacc.Bacc("TRN2")
    # const AP for the activation bias (+0.5); only 0.0/1.0 are pre-registered.
    # The memset runs in the preamble, ~7us before the first activation that
    # reads it; gpsimd program order also puts it before any gpsimd DMA.
    const_half = nc.alloc_sbuf_tensor("const-float32-0.5", [128, 1], f32)
    nc.vector.memset(const_half.ap(), 0.5)
    nc.const_aps.aps[(f32, 0.5)] = const_half.ap()
    # Chunk-blocked SBUF images (host does the shuffles):
    #   xQ[p, NKC*off(sc) + k*w + j] = x[b][off+j, k*128+p]
    #   wQ[fc, p, k*256 + e]         = W row (c*DH + fc*128 + e%128), col k*128+p
    #   outQ[DH*off(sc) + (fc*128+p)*w + j] = h[off+j, fc*128+p]
    xQ = nc.dram_tensor("xQ", [KC, NKC * S], f16, kind="ExternalInput")
    wQ = nc.dram_tensor("wQ", [NFC, KC, NKC * W2], f16, kind="ExternalInput")
    outQ = nc.dram_tensor("outQ", [DH * S], f16, kind="ExternalOutput")

    with tile.TileContext(nc) as tc:
        with (
            tc.tile_pool(name="w", bufs=1) as wpool,
            tc.tile_pool(name="x", bufs=3) as xpool,
            tc.tile_pool(name="ew", bufs=3) as epool,
            tc.tile_pool(name="h", bufs=2) as hpool,
            tc.tile_pool(name="ps", bufs=4, space="PSUM") as pspool,
        ):
            wts = []
            for fc in range(NFC):
                wtf = wpool.tile([KC, NKC * W2], f16, tag=f"w{fc}")
                wts.append(wtf)

            # Ramp notes (measured): DMA pieces with <2KB rows crawl; each
            # queue pays ~2us startup + ~0.6us issue per DMA; scalar's queue
            # is separate from sync's and starts early; gpsimd's starts
            # ~1.7us later; a backlogged queue moves ~350GB/s.
            w0 = WIDTHS[0]
            xt0 = xpool.tile([KC, NKC * w0], f16, tag="xt")

            def wpiece(eng, fc, k0, k1):  # k-range of W fc (quads: 2KB rows)
                eng.dma_start(wts[fc][:, k0 * W2:k1 * W2],
                              wQ[fc, :, k0 * W2:k1 * W2])

            def xpair(eng, k):
                eng.dma_start(xt0[:, k * w0:(k + 2) * w0],
                              xQ[:, k * w0:(k + 2) * w0])

            # Chunks 0-1 run fc-STAGED (fc0/fc1 over both chunks, then
            # fc2/fc3), so the early window only needs half of W: the ramp
            # is never W-supply-bound. Early needs: x0 + Wfc0 now, Wfc1 at
            # +3.5us, xt1 at +7us; everything else has >10us of slack.
            for k in range(0, NKC, 2):
                xpair(nc.sync, k)
            wpiece(nc.scalar, 0, 0, 4)
            wpiece(nc.scalar, 0, 4, 8)
            wpiece(nc.scalar, 1, 0, 8)
            xt1 = xpool.tile([KC, NKC * WIDTHS[1]], f16, tag="xt")
            nc.sync.dma_start(xt1[:], xQ[:, NKC * w0:NKC * (w0 + WIDTHS[1])])
            wpiece(nc.sync, 2, 0, 8)
            wpiece(nc.gpsimd, 3, 0, 8)

            hprev = [None] * NFC
            offs = []
            o = 0
            for w in WIDTHS:
                offs.append(o)
                o += w
            xts = {0: xt0, 1: xt1}

            def sched():
                yield from ((0, 0), (0, 1), (1, 0), (1, 1),
                            (0, 2), (0, 3), (1, 2), (1, 3))
                for sc_ in range(2, len(WIDTHS) - 1):
                    for fc_ in range(NFC):
                        yield (sc_, fc_)

            for sc, fc in sched():
                width, off = WIDTHS[sc], offs[sc]
                if sc not in xts:
                    xt = xpool.tile([KC, NKC * width], f16, tag="xt")
                    base = NKC * off
                    nc.sync.dma_start(xt[:], xQ[:, base:base + NKC * width])
                    xts[sc] = xt
                xt = xts[sc]
                ph = pspool.tile([FC, width], f32, tag="ph")
                pg = pspool.tile([FC, width], f32, tag="pg")
                # h/g interleaved per k: same steady cadence as split loops,
                # but halves the W/x arrival-rate requirement at boundaries
                for k in range(NKC):
                    nc.tensor.matmul(
                        ph[:], wts[fc][:, k * W2:k * W2 + FC],
                        xt[:, k * width:(k + 1) * width],
                        start=(k == 0), stop=(k == NKC - 1),
                    )
                    nc.tensor.matmul(
                        pg[:], wts[fc][:, k * W2 + FC:(k + 1) * W2],
                        xt[:, k * width:(k + 1) * width],
                        start=(k == 0), stop=(k == NKC - 1),
                    )
                st = epool.tile([FC, width], f16, tag="s")
                hp = epool.tile([FC, width], f16, tag="hp")
                zt = epool.tile([FC, width], f16, tag="z")
                ct = epool.tile([FC, width], f16, tag="c")
                gt = epool.tile([FC, width], f16, tag="g")
                ut = epool.tile([FC, width], f16, tag="u")
                # scalar engine drains psum: s/hp from ph, z/c from pg
                nc.scalar.activation(st[:], ph[:], AF.Sigmoid)
                nc.scalar.activation(hp[:], ph[:], AF.Identity, bias=0.5)
                nc.scalar.activation(zt[:], pg[:], AF.Sigmoid)
                nc.scalar.activation(ct[:], pg[:], AF.Sigmoid, scale=-1.0)
                # DVE in 2x fp16 mode: g = max(h+0.5, sigmoid(h)); u = z*g
                nc.vector.tensor_tensor(gt[:], hp[:], st[:], op=OP.max)
                nc.vector.tensor_mul(ut[:], zt[:], gt[:])
                ht = hpool.tile([FC, width], f16, tag=f"h{fc}")
                init = 0.0 if sc == 0 else hprev[fc][:, WIDTHS[sc - 1] - 1:WIDTHS[sc - 1]]
                nc.vector.tensor_tensor_scan(
                    ht[:], ct[:], ut[:], init, op0=OP.mult, op1=OP.add
                )
                hprev[fc] = ht
                dst = outQ[
                    DH * off + fc * FC * width:
                    DH * off + (fc + 1) * FC * width
                ].rearrange("(p w) -> p w", p=FC)
                # alternate queues so the final chunks' outputs drain in
                # parallel at the end instead of serializing on one ring
                oeng = nc.gpsimd if sc % 2 == 0 else nc.sync
                oeng.dma_start(dst, ht[:])

            # ---- last chunk ----
            for sc in (len(WIDTHS) - 1,):
                width, off = WIDTHS[sc], offs[sc]
                xt = xpool.tile([KC, NKC * width], f16, tag="xt")
                base = NKC * off
                nc.sync.dma_start(xt[:], xQ[:, base:base + NKC * width])
                if True:
                    # Last chunk (width 128): the tail after the final matmul
                    # is dominated by per-op fixed costs (~0.3-0.5us each).
                    # Group fc0-2 into one shared psum pair with triple-width
                    # elementwise ops (overlapping fc3's matmuls), leaving
                    # only fc3's minimal narrow chain after the last matmul.
                    pw = WIDTHS[sc - 1]
                    for gfcs in ((0, 1), (2,), (3,)):
                        gw = len(gfcs) * width
                        phm = pspool.tile([FC, gw], f32, tag="ph")
                        pgm = pspool.tile([FC, gw], f32, tag="pg")
                        for gi, fc in enumerate(gfcs):
                            sl = slice(gi * width, (gi + 1) * width)
                            for k in range(NKC):
                                nc.tensor.matmul(
                                    phm[:, sl], wts[fc][:, k * W2:k * W2 + FC],
                                    xt[:, k * width:(k + 1) * width],
                                    start=(k == 0), stop=(k == NKC - 1),
                                )
                                nc.tensor.matmul(
                                    pgm[:, sl], wts[fc][:, k * W2 + FC:(k + 1) * W2],
                                    xt[:, k * width:(k + 1) * width],
                                    start=(k == 0), stop=(k == NKC - 1),
                                )
                        st = epool.tile([FC, gw], f16, tag="s")
                        hp = epool.tile([FC, gw], f16, tag="hp")
                        zt = epool.tile([FC, gw], f16, tag="z")
                        ct = epool.tile([FC, gw], f16, tag="c")
                        gt = epool.tile([FC, gw], f16, tag="g")
                        ut = epool.tile([FC, gw], f16, tag="u")
                        nc.scalar.activation(st[:], phm[:], AF.Sigmoid)
                        nc.scalar.activation(hp[:], phm[:], AF.Identity, bias=0.5)
                        nc.scalar.activation(zt[:], pgm[:], AF.Sigmoid)
                        nc.scalar.activation(ct[:], pgm[:], AF.Sigmoid, scale=-1.0)
                        nc.vector.tensor_tensor(gt[:], hp[:], st[:], op=OP.max)
                        nc.vector.tensor_mul(ut[:], zt[:], gt[:])
                        for gi, fc in enumerate(gfcs):
                            sl = slice(gi * width, (gi + 1) * width)
                            ht = hpool.tile([FC, width], f16, tag=f"h{fc}")
                            nc.vector.tensor_tensor_scan(
                                ht[:], ct[:, sl], ut[:, sl],
                                hprev[fc][:, pw - 1:pw],
                                op0=OP.mult, op1=OP.add,
                            )
                            dst = outQ[
                                DH * off + fc * FC * width:
                                DH * off + (fc + 1) * FC * width
                            ].rearrange("(p w) -> p w", p=FC)
                            oeng = nc.gpsimd if fc % 2 == 0 else nc.sync
                            oeng.dma_start(dst, ht[:])

    nc.compile()
    return nc


def _prep_in_maps(x: np.ndarray, W_hg: np.ndarray):
    x = np.asarray(x, dtype=np.float32)
    W_hg = np.asarray(W_hg, dtype=np.float32)
    xQs = []
    for b in range(B):
        xb = x[b].astype(np.float16)                        # [S, D]
        xq = np.empty((KC, NKC * S), dtype=np.float16)
        o = 0
        for w in WIDTHS:
            blk = xb[o:o + w].T.reshape(NKC, KC, w).transpose(1, 0, 2)
            xq[:, NKC * o:NKC * (o + w)] = blk.reshape(KC, NKC * w)
            o += w
        xQs.append(xq)
    wQs = []
    for c in range(2):
        wq = np.empty((NFC, KC, NKC * W2), dtype=np.float16)
        for fc in range(NFC):
            rows_h = W_hg[c * DH + fc * FC:c * DH + (fc + 1) * FC]      # [FC, D]
            rows_g = W_hg[D + c * DH + fc * FC:D + c * DH + (fc + 1) * FC]
            wfc = np.empty((D, W2), dtype=np.float16)
            wfc[:, 0:FC] = rows_h.T
            wfc[:, FC:W2] = rows_g.T
            wq[fc] = wfc.reshape(NKC, KC, W2).transpose(1, 0, 2).reshape(KC, NKC * W2)
        wQs.append(wq)
    return [{"xQ": xQs[core // 2], "wQ": wQs[core % 2]} for core in range(N_CORES)]


def _get_runner():
    """Build the Bass module once and cache a compiled jax callable for it."""
    if "runner" in _CACHE:
        return _CACHE["runner"]

    import jax
    from jax.experimental.shard_map import shard_map
    from jax.sharding import Mesh, PartitionSpec
    from concourse import bass2jax

    if "nc" not in _CACHE:
        _CACHE["nc"] = _build()
    nc = _CACHE["nc"]
    bass2jax.install_neuronx_cc_hook()

    in_names = ["xQ", "wQ"]
    out_name = "outQ"
    out_shape, out_dtype = (DH * S,), np.float16
    partition_name = nc.partition_id_tensor.name if nc.partition_id_tensor else None

    def _body(xQ, wQ, zout):
        operands = [xQ, wQ, zout]
        if partition_name is not None:
            operands.append(bass2jax.partition_id_tensor())
        outs = bass2jax._bass_exec_p.bind(
            *operands,
            out_avals=(jax.core.ShapedArray(out_shape, out_dtype),),
            in_names=tuple(in_names + [out_name] + ([partition_name] if partition_name else [])),
            out_names=(out_name,),
            lowering_input_output_aliases=(),
            sim_require_finite=True,
            sim_require_nnan=True,
            nc=nc,
        )
        return tuple(outs)

    devices = jax.devices()[:N_CORES]
    mesh = Mesh(np.asarray(devices), ("core",))
    sharded = jax.jit(
        shard_map(
            _body, mesh=mesh,
            in_specs=(PartitionSpec("core"),) * 3,
            out_specs=(PartitionSpec("core"),),
            check_rep=False,
        ),
        donate_argnums=(2,),
        keep_unused=True,
    )

    def run(in_maps):
        concat_x = np.concatenate([m["xQ"] for m in in_maps], axis=0)
        concat_w = np.concatenate([m["wQ"] for m in in_maps], axis=0)
        zeros = np.zeros((N_CORES * DH * S,), np.float16)
        (out_arr,) = sharded(concat_x, concat_w, zeros)
        return np.asarray(out_arr).reshape(N_CORES, DH * S)

    _CACHE["runner"] = run
    return run


def kernel(x: np.ndarray, W_hg: np.ndarray) -> np.ndarray:
    run = _get_runner()
    in_maps = _prep_in_maps(x, W_hg)
    outs = run(in_maps)

    out = np.empty((B, S, D), dtype=np.float32)
    for core in range(N_CORES):
        b, c = core // 2, core % 2
        flat = outs[core]
        o = 0
        for w in WIDTHS:
            blk = flat[DH * o:DH * (o + w)].reshape(DH, w)
            out[b, o:o + w, c * DH:(c + 1) * DH] = blk.T.astype(np.float32)
            o += w
    return out
